# revision 38
# baseline (speedup 1.0000x reference)
"""nn_AdaptiveGraphLayer Trainium2 kernel (8 NeuronCores, SPMD).

Sharding: each core owns N/8 = 512 query rows for all H=4 heads.
 - x (node features) replicated -> every core computes K/V for all nodes.
 - adjacency mask built on host, sharded and transposed to [4096,512] (0/1
   multiplicative, kc-chunked).
 - attn is produced on device in a k-major blocked layout
   [H, 32, 128, 512] = [h, k-chunk, k, q] (fully normalized values); the
   host gather permutes it back to [H, 512, 4096]. out rows are f32.
   No device collectives.

Per-head device pipeline (single scores pass in the transposed layout):
  pass1: scores^T chunks via PE (K extended to 65 with a ones column ->
         row 64 of U~ accumulates the masked-exp row sums for free);
         ACT exp straight from PSUM (|scores| < ~4, safe unmasked);
         DVE/GPSIMD mask-multiply (2x bf16) into a per-head em buffer;
         PE-accumulate U~ = [V_h | 1]^T @ em.
  C:     evacuate U~, PE-transpose, reciprocal of sums -> rinv;
         scale the V-aggregation; build a broadcast rinv row.
  pass2: normalize em in place (DVE 2x bf16) and DMA the attn blocks out.
  D:     residual + LayerNorm (bn_stats/bn_aggr), affine, DMA out rows.
"""

import numpy as np

B, N, F, H, HD = 1, 4096, 256, 4, 64
NCORES = 8
NQ = N // NCORES          # 512 query rows per core
P = 128
HD1 = HD + 1              # V plus the ones column
KC = 32                   # key chunks of 128
LN_EPS = 1e-5
GPS_MOD = 0               # every GPS_MOD-th mask-mult tile on GPSIMD (0=off)

TRACE = False             # set True (with ntff shim installed) to profile
LAST_EXEC_NS = None
LAST_RESULTS = None

_BUILT = None


def _build():
    from contextlib import ExitStack

    import concourse.bass as bass
    import concourse.bacc as bacc
    import concourse.mybir as mybir
    from concourse.tile import TileContext
    from concourse.masks import make_identity

    dt = mybir.dt
    f32, bf16 = dt.float32, dt.bfloat16
    AF = mybir.ActivationFunctionType
    OP = mybir.AluOpType

    nc = bacc.Bacc("TRN2", target_bir_lowering=False, debug=False,
                   num_devices=NCORES)

    def din(name, shape, dtype):
        return nc.declare_dram_parameter(name, list(shape), dtype, isOutput=False)

    def dout(name, shape, dtype):
        return nc.declare_dram_parameter(name, list(shape), dtype, isOutput=True)

    xT = din("xT", [2, P, N], bf16)          # x^T [f_in, node], f_in-chunked
    xqT = din("xqT", [2, P, NQ], bf16)       # this core's q columns of x^T
    xr = din("xr", [NQ, F], f32)             # residual rows (f32)
    wqT = din("wqT", [2, P, F], bf16)        # Wq.T [f_in, out], f_in-chunked
    wkT = din("wkT", [2, P, F], bf16)
    wvT = din("wvT", [2, P, F], bf16)
    bq8 = din("bq8", [2, P, 1], f32)         # bq / sqrt(hd)
    bkb = din("bkb", [2, P, 1], f32)
    bvb = din("bvb", [1, F], bf16)
    lng = din("lng", [1, F], f32)
    lnb = din("lnb", [1, F], f32)
    mmulT = din("mmulT", [KC, P, NQ], bf16)  # 0/1 mask^T, kc-chunked
    attn_d = dout("attn", [H, KC, P, NQ], bf16)   # [h, kc, k, q] blocks
    out_d = dout("outp", [NQ, F], f32)
    rr_dram = nc.dram_tensor("rr_scratch", [H, NQ], bf16)

    with TileContext(nc) as tc, ExitStack() as ctx:
        const = ctx.enter_context(tc.tile_pool(name="const", bufs=1))
        big = ctx.enter_context(tc.tile_pool(name="big", bufs=1))
        emh = ctx.enter_context(tc.tile_pool(name="emh", bufs=8))
        wk_b = ctx.enter_context(tc.tile_pool(name="wk_b", bufs=3))
        wk_d = ctx.enter_context(tc.tile_pool(name="wk_d", bufs=1))
        pmm = ctx.enter_context(tc.tile_pool(name="pmm", bufs=3, space="PSUM"))
        pu = ctx.enter_context(tc.tile_pool(name="pu", bufs=2, space="PSUM"))

        # ---- constants / parameters to SBUF ----
        ident = const.tile([HD1, HD1], f32)
        make_identity(nc, ident[:])
        ones1 = const.tile([1, P], bf16)
        nc.vector.memset(ones1[:], 1.0)
        ones_pad = const.tile([P, P], bf16)
        nc.vector.memset(ones_pad[:], 1.0)

        xT_sb = big.tile([P, 2, N], bf16)
        xqT_sb = big.tile([P, 2, NQ], bf16)
        KT_sb = big.tile([P, 2, N], bf16)
        QT_sb = big.tile([P, 2, NQ], bf16)
        V_sb = big.tile([P, KC, H, HD1], bf16)     # V plus ones column
        mmulT_sb = big.tile([P, KC, NQ], bf16)
        nc.vector.memset(V_sb[:, :, :, HD:HD1], 1.0)
        # small, projection-critical loads first; big masks last
        bq8_sb = const.tile([P, 2], f32)
        bk_sb = const.tile([P, 2], f32)
        for mc in range(2):
            nc.sync.dma_start(out=bq8_sb[:, mc:mc + 1], in_=bq8[mc])
            nc.sync.dma_start(out=bk_sb[:, mc:mc + 1], in_=bkb[mc])

        def bcast(dram_ap):
            return bass.AP(tensor=dram_ap.tensor, offset=dram_ap.offset,
                           ap=[[0, P]] + list(dram_ap.ap[1:]))

        bv_bc = const.tile([P, F], bf16)
        nc.sync.dma_start(out=bv_bc[:], in_=bcast(bvb[:]))
        g_bc = const.tile([P, F], f32)
        b_bc = const.tile([P, F], f32)
        nc.sync.dma_start(out=g_bc[:], in_=bcast(lng[:]))
        nc.sync.dma_start(out=b_bc[:], in_=bcast(lnb[:]))

        xr_sb = const.tile([P, 4, F], f32)
        for qc in range(4):
            nc.sync.dma_start(out=xr_sb[:, qc, :], in_=xr[qc * P:(qc + 1) * P, :])

        w_sb = {}
        for nm, t in (("q", wqT), ("k", wkT), ("v", wvT)):
            w_sb[nm] = big.tile([P, 2, F], bf16, tag=f"w{nm}", name=f"w{nm}_sb")
            for kc in range(2):
                nc.sync.dma_start(out=w_sb[nm][:, kc, :], in_=t[kc])
        for kc in range(2):
            nc.sync.dma_start(out=xqT_sb[:, kc, :], in_=xqT[kc])
            nc.sync.dma_start(out=xT_sb[:, kc, :], in_=xT[kc])
        for kc in range(KC):
            nc.sync.dma_start(out=mmulT_sb[:, kc, :], in_=mmulT[kc])

        eps_sb = const.tile([P, 1], f32)
        nc.vector.memset(eps_sb[:], LN_EPS)
        rinv_sb = const.tile([P, H * 4], f32)
        out_sb = const.tile([P, 4, F], f32)
        srow = const.tile([1, H, NQ], f32)
        rr32 = const.tile([1, H, NQ], f32)
        rrbf = const.tile([1, H, NQ], bf16)

        # ---- projections ----
        for mc in range(2):
            ps = pmm.tile([P, 1024], f32, tag="mm", name="ps_q")
            for kc in range(2):
                nc.tensor.matmul(ps[:, 0:NQ],
                                 w_sb["q"][:, kc, mc * P:(mc + 1) * P],
                                 xqT_sb[:, kc, :],
                                 start=(kc == 0), stop=(kc == 1))
            nc.vector.tensor_scalar(out=QT_sb[:, mc, :], in0=ps[:, 0:NQ],
                                    scalar1=1.0 / np.sqrt(HD),
                                    scalar2=bq8_sb[:, mc:mc + 1],
                                    op0=OP.mult, op1=OP.add)
        def emit_kt_proj(mc, n2):
            ps = pmm.tile([P, 1024], f32, tag="mm", name="ps_k")
            for j in range(2):
                n0 = n2 * 1024 + j * 512
                for kc in range(2):
                    nc.tensor.matmul(ps[:, j * 512:(j + 1) * 512],
                                     w_sb["k"][:, kc, mc * P:(mc + 1) * P],
                                     xT_sb[:, kc, n0:n0 + 512],
                                     start=(kc == 0), stop=(kc == 1))
            nc.scalar.activation(
                out=KT_sb[:, mc, n2 * 1024:(n2 + 1) * 1024], in_=ps[:],
                func=AF.Identity, bias=bk_sb[:, mc:mc + 1], scale=1.0)

        def emit_v_proj(v4):
            # four V chunks (512 nodes) per PSUM tile
            ps = pmm.tile([P, 1024], f32, tag="mm", name="ps_v")
            for c in range(4):
                nc32 = 4 * v4 + c
                for kc in range(2):
                    nc.tensor.matmul(ps[:, c * F:(c + 1) * F],
                                     xT_sb[:, kc, nc32 * P:(nc32 + 1) * P],
                                     w_sb["v"][:, kc, :],
                                     start=(kc == 0), stop=(kc == 1))
            bv_ap = bv_bc[:]
            bv4 = bass.AP(tensor=bv_ap.tensor, offset=bv_ap.offset,
                          ap=[list(bv_ap.ap[0]), [0, 4], [HD, H], [1, HD]])
            nc.vector.scalar_tensor_tensor(
                out=V_sb[:, 4 * v4:4 * v4 + 4, :, 0:HD],
                in0=ps[:].rearrange("p (c h d) -> p c h d", c=4, h=H),
                scalar=0.0, in1=bv4, op0=OP.bypass, op1=OP.add)

        for mc in range(2):
            for n2 in range(4):
                emit_kt_proj(mc, n2)
        for v4 in range(8):
            emit_v_proj(v4)

        def qk_slice(t, h, lo, sz):
            return t[(h % 2) * HD:(h % 2 + 1) * HD, h // 2, lo:lo + sz]

        def emit_pass2_chunk(h, emc, rb, c8):
            # normalize one em chunk in place and DMA its attn blocks out
            rb_ap = rb[:]
            rb_rep = bass.AP(tensor=rb_ap.tensor, offset=rb_ap.offset,
                             ap=[list(rb_ap.ap[0]), [0, 2], list(rb_ap.ap[1])])
            for t4 in range(4):
                nc.vector.tensor_tensor(
                    out=emc[:, 2 * t4:2 * t4 + 2, :],
                    in0=emc[:, 2 * t4:2 * t4 + 2, :],
                    in1=rb_rep, op=OP.mult)
            nc.sync.dma_start(
                out=attn_d[h, 8 * c8:8 * (c8 + 1)].rearrange("c p q -> p c q"),
                in_=emc[:])

        prev = None       # (h, em_chunks, rb) of the previous head
        for h in range(H):
            # ---- pass 1: exp(scores^T) * mask -> em buffer + U~ accumulation
            # Software-pipelined: the U matmuls lag the score matmuls by
            # PIPE tiles so the PE never stalls on a fresh mask-multiply,
            # and the previous head's pass-2 chunks are interleaved in.
            PIPE = 3
            if prev is not None:
                for c8 in range(4):
                    emit_pass2_chunk(prev[0], prev[1][c8], prev[2], c8)
            up = pu.tile([HD1, NQ], f32, tag="u", name="up")
            em_chunks = []
            tiles = []    # per kc-pair: emc tile + slice index

            def emit_u(i):
                emc_i, t4_i = tiles[i]
                for j in range(2):
                    nc.tensor.matmul(up[:],
                                     V_sb[:, 2 * i + j, h, :],
                                     emc_i[:, 2 * t4_i + j, :],
                                     start=(2 * i + j == 0),
                                     stop=(2 * i + j == KC - 1))

            for i in range(16):          # kc pairs
                c8, t4 = i // 4, i % 4
                if t4 == 0:
                    emc = emh.tile([P, 8, NQ], bf16, tag="emh", name="emc")
                    em_chunks.append(emc)
                kc0 = 2 * i
                ps = pmm.tile([P, 1024], f32, tag="mm", name="ps_b")
                for j in range(2):
                    nc.tensor.matmul(ps[:, j * NQ:(j + 1) * NQ],
                                     qk_slice(KT_sb, h, (kc0 + j) * P, P),
                                     qk_slice(QT_sb, h, 0, NQ),
                                     start=True, stop=True)
                et = wk_b.tile([P, 1024], bf16, tag="et", name="et")
                nc.scalar.activation(out=et[:], in_=ps[:], func=AF.Exp)
                eng = nc.gpsimd if (GPS_MOD and i % GPS_MOD == 0) \
                    else nc.vector
                eng.tensor_tensor(
                    out=emc[:, 2 * t4:2 * t4 + 2, :],
                    in0=et[:].rearrange("p (j q) -> p j q", j=2),
                    in1=mmulT_sb[:, kc0:kc0 + 2, :], op=OP.mult)
                tiles.append((emc, t4))
                if i >= PIPE:
                    emit_u(i - PIPE)
            for i in range(16 - PIPE, 16):
                emit_u(i)
            # ---- C: evacuate U~, transpose, rinv, scale V-aggregation
            us = wk_b.tile([HD1, NQ], f32, tag="us", name="us")
            nc.scalar.activation(out=us[:], in_=up[:], func=AF.Copy)
            tp = pu.tile([P, 4, HD1], f32, tag="u", name="tp")
            for qc in range(4):
                nc.tensor.transpose(tp[:, qc, :],
                                    us[:, qc * P:(qc + 1) * P], ident[:])
            for qc in range(4):
                idx = h * 4 + qc
                nc.vector.reciprocal(out=rinv_sb[:, idx:idx + 1],
                                     in_=tp[:, qc, HD:HD1])
                nc.vector.tensor_scalar_mul(
                    out_sb[:, qc, h * HD:(h + 1) * HD], tp[:, qc, 0:HD],
                    rinv_sb[:, idx:idx + 1])
            # rinv broadcast row: invert the sums row in place (stays on
            # partition 64), cast to bf16, then replicate across partitions
            # with a K=1 ones matmul on the PE
            nc.sync.dma_start(out=srow[:, h, :], in_=us[HD:HD1, :])
            nc.vector.reciprocal(out=rr32[:, h, :], in_=srow[:, h, :])
            nc.vector.tensor_copy(rrbf[:, h, :], rr32[:, h, :])
            rb = wk_b.tile([P, NQ], bf16, tag="rb", name="rb")
            nc.sync.dma_start(out=rr_dram[h:h + 1, :], in_=rrbf[:, h, :])
            nc.sync.dma_start(out=rb[:], in_=bcast(rr_dram[h:h + 1, :]))
            prev = (h, em_chunks, rb)

        # last head's pass 2
        for c8 in range(4):
            emit_pass2_chunk(prev[0], prev[1][c8], prev[2], c8)

        # ---- phase D: residual + layernorm + affine ----
        for qc in range(4):
            y = wk_d.tile([P, F], f32, tag="y", name="y")
            nc.vector.tensor_tensor(out=y[:], in0=out_sb[:, qc, :],
                                    in1=xr_sb[:, qc, :], op=OP.add)
            st = wk_d.tile([P, 6], f32, tag="st", name="st")
            nc.vector.bn_stats(out=st[:], in_=y[:])
            mv = wk_d.tile([P, 2], f32, tag="mv", name="mv")
            nc.vector.bn_aggr(out=mv[:], in_=st[:])
            sd = wk_d.tile([P, 1], f32, tag="sd", name="sd")
            nc.scalar.activation(out=sd[:], in_=mv[:, 1:2], func=AF.Sqrt,
                                 bias=eps_sb[:], scale=1.0)
            rs = wk_d.tile([P, 1], f32, tag="rs", name="rs")
            nc.vector.reciprocal(out=rs[:], in_=sd[:])
            yc = wk_d.tile([P, F], f32, tag="yc", name="yc")
            nc.vector.tensor_scalar(out=yc[:], in0=y[:],
                                    scalar1=mv[:, 0:1], scalar2=rs[:],
                                    op0=OP.subtract, op1=OP.mult)
            yg = wk_d.tile([P, F], f32, tag="yg", name="yg")
            nc.vector.tensor_tensor(out=yg[:], in0=yc[:], in1=g_bc[:],
                                    op=OP.mult)
            yo = wk_d.tile([P, F], f32, tag="yo", name="yo")
            nc.vector.tensor_tensor(out=yo[:], in0=yg[:], in1=b_bc[:],
                                    op=OP.add)
            nc.sync.dma_start(out=out_d[qc * P:(qc + 1) * P, :], in_=yo[:])

    nc.finalize()
    return nc


def _shard_inputs(inputs):
    import ml_dtypes
    bf = ml_dtypes.bfloat16

    x = np.asarray(inputs["x"], np.float32)
    ei = np.asarray(inputs["edge_index"], np.int64)
    Wq = np.asarray(inputs["Wq"], np.float32)
    bq = np.asarray(inputs["bq"], np.float32)
    Wk = np.asarray(inputs["Wk"], np.float32)
    bk = np.asarray(inputs["bk"], np.float32)
    Wv = np.asarray(inputs["Wv"], np.float32)
    bv = np.asarray(inputs["bv"], np.float32)
    ln_g = np.asarray(inputs["ln_g"], np.float32)
    ln_b = np.asarray(inputs["ln_b"], np.float32)

    adj = np.zeros((N, N), np.bool_)
    adj[ei[0], ei[1]] = True

    x0 = x[0]                                     # [N, F]
    xT = np.ascontiguousarray(x0.T)               # [F, N]

    def b16(a):
        return np.ascontiguousarray(a).astype(bf)

    xT_b = b16(xT).reshape(2, P, N)
    w = {nm: b16(W.T).reshape(2, P, F)
         for nm, W in (("q", Wq), ("k", Wk), ("v", Wv))}
    shared = {
        "xT": xT_b,
        "wqT": w["q"], "wkT": w["k"], "wvT": w["v"],
        "bq8": (bq / np.sqrt(HD)).astype(np.float32).reshape(2, P, 1),
        "bkb": bk.astype(np.float32).reshape(2, P, 1),
        "bvb": bv.reshape(1, F).astype(bf),
        "lng": ln_g.reshape(1, F).astype(np.float32),
        "lnb": ln_b.reshape(1, F).astype(np.float32),
    }
    in_maps = []
    for c in range(NCORES):
        rows = slice(c * NQ, (c + 1) * NQ)
        a = adj[rows]                             # [NQ, N]
        m = {
            "xqT": b16(xT[:, rows]).reshape(2, P, NQ),
            "xr": np.ascontiguousarray(x0[rows]).astype(np.float32),
            "mmulT": np.ascontiguousarray(a.T).astype(bf).reshape(KC, P, NQ),
        }
        m.update(shared)
        in_maps.append(m)
    return in_maps


def kernel(**inputs):
    global _BUILT, LAST_EXEC_NS, LAST_RESULTS
    from concourse.bass_utils import run_bass_kernel_spmd

    if _BUILT is None:
        _BUILT = _build()
    nc = _BUILT

    in_maps = _shard_inputs(inputs)
    res = run_bass_kernel_spmd(nc, in_maps, core_ids=list(range(NCORES)),
                               trace=TRACE)
    LAST_EXEC_NS = res.exec_time_ns
    LAST_RESULTS = res

    attn_full = np.empty((1, H, N, N), np.float32)
    out_full = np.empty((1, N, F), np.float32)
    for c in range(NCORES):
        rows = slice(c * NQ, (c + 1) * NQ)
        shard = np.asarray(res.results[c]["attn"])        # [H, KC, P, NQ]
        shard = shard.reshape(H, N, NQ).astype(np.float32)
        attn_full[0, :, rows, :] = shard.transpose(0, 2, 1)
        out_full[0, rows, :] = np.asarray(
            res.results[c]["outp"]).astype(np.float32)
    return out_full, attn_full


# revision 39
# speedup vs baseline: 1.0787x; 1.0787x over previous
"""nn_AdaptiveGraphLayer Trainium2 kernel (8 NeuronCores, SPMD).

Sharding: each core owns N/8 = 512 query rows for all H=4 heads.
 - x (node features) replicated -> every core computes K/V for all nodes.
 - adjacency mask built on host, sharded and transposed to [4096,512] (0/1
   multiplicative, kc-chunked).
 - attn is produced on device in a k-major blocked layout
   [H, 32, 128, 512] = [h, k-chunk, k, q] (fully normalized values); the
   host gather permutes it back to [H, 512, 4096]. out rows are f32.
   No device collectives.

Per-head device pipeline (single scores pass in the transposed layout):
  pass1: scores^T chunks via PE (K extended to 65 with a ones column ->
         row 64 of U~ accumulates the masked-exp row sums for free);
         ACT exp straight from PSUM (|scores| < ~4, safe unmasked);
         DVE mask-multiply (2x bf16) into a per-head em buffer;
         PE-accumulate U~ = [V_h | 1]^T @ em.
  C:     evacuate U~, PE-transpose, reciprocal of sums -> rinv;
         scale the V-aggregation; build a broadcast rinv row.
  pass2: normalize em in place (DVE 2x bf16) and DMA the attn blocks out.
  D:     residual + LayerNorm (bn_stats/bn_aggr), affine, DMA out rows.
"""

import numpy as np

B, N, F, H, HD = 1, 4096, 256, 4, 64
NCORES = 8
NQ = N // NCORES          # 512 query rows per core
P = 128
HD1 = HD + 1              # V plus the ones column
KC = 32                   # key chunks of 128
LN_EPS = 1e-5
GPS_MOD = 0               # every GPS_MOD-th mask-mult tile on GPSIMD (0=off)

TRACE = False             # set True (with ntff shim installed) to profile
LAST_EXEC_NS = None
LAST_RESULTS = None

_BUILT = None


def _build():
    from contextlib import ExitStack

    import concourse.bass as bass
    import concourse.bacc as bacc
    import concourse.mybir as mybir
    from concourse.tile import TileContext
    from concourse.masks import make_identity

    dt = mybir.dt
    f32, bf16 = dt.float32, dt.bfloat16
    AF = mybir.ActivationFunctionType
    OP = mybir.AluOpType

    nc = bacc.Bacc("TRN2", target_bir_lowering=False, debug=False,
                   num_devices=NCORES)

    def din(name, shape, dtype):
        return nc.declare_dram_parameter(name, list(shape), dtype, isOutput=False)

    def dout(name, shape, dtype):
        return nc.declare_dram_parameter(name, list(shape), dtype, isOutput=True)

    xT = din("xT", [2, P, N], bf16)          # x^T [f_in, node], f_in-chunked
    xqT = din("xqT", [2, P, NQ], bf16)       # this core's q columns of x^T
    xr = din("xr", [NQ, F], f32)             # residual rows (f32)
    wqT = din("wqT", [2, P, F], bf16)        # Wq.T [f_in, out], f_in-chunked
    wkT = din("wkT", [2, P, F], bf16)
    wvT = din("wvT", [2, P, F], bf16)
    bq8 = din("bq8", [2, P, 1], f32)         # bq / sqrt(hd)
    bkb = din("bkb", [2, P, 1], f32)
    bvb = din("bvb", [1, F], bf16)
    lng = din("lng", [1, F], f32)
    lnb = din("lnb", [1, F], f32)
    mmulT = din("mmulT", [KC, P, NQ], bf16)  # 0/1 mask^T, kc-chunked
    attn_d = dout("attn", [H, KC, P, NQ], bf16)   # [h, kc, k, q] blocks
    out_d = dout("outp", [NQ, F], f32)
    rr_dram = nc.dram_tensor("rr_scratch", [H, NQ], bf16)

    with TileContext(nc) as tc, ExitStack() as ctx:
        const = ctx.enter_context(tc.tile_pool(name="const", bufs=1))
        big = ctx.enter_context(tc.tile_pool(name="big", bufs=1))
        emh = ctx.enter_context(tc.tile_pool(name="emh", bufs=8))
        wk_b = ctx.enter_context(tc.tile_pool(name="wk_b", bufs=3))
        wk_d = ctx.enter_context(tc.tile_pool(name="wk_d", bufs=1))
        pmm = ctx.enter_context(tc.tile_pool(name="pmm", bufs=3, space="PSUM"))
        pu = ctx.enter_context(tc.tile_pool(name="pu", bufs=2, space="PSUM"))

        # ---- constants / parameters to SBUF ----
        ident = const.tile([HD1, HD1], f32)
        make_identity(nc, ident[:])
        ones1 = const.tile([1, P], bf16)
        nc.vector.memset(ones1[:], 1.0)
        ones_pad = const.tile([P, P], bf16)
        nc.vector.memset(ones_pad[:], 1.0)

        xT_sb = big.tile([P, 2, N], bf16)
        xqT_sb = big.tile([P, 2, NQ], bf16)
        KT_sb = big.tile([P, 2, N], bf16)
        QT_sb = big.tile([P, 2, NQ], bf16)
        V_sb = big.tile([P, KC, H, HD1], bf16)     # V plus ones column
        mmulT_sb = big.tile([P, KC, NQ], bf16)
        nc.vector.memset(V_sb[:, :, :, HD:HD1], 1.0)
        # small, projection-critical loads first; big masks last
        bq8_sb = const.tile([P, 2], f32)
        bk_sb = const.tile([P, 2], f32)
        for mc in range(2):
            nc.sync.dma_start(out=bq8_sb[:, mc:mc + 1], in_=bq8[mc])
            nc.sync.dma_start(out=bk_sb[:, mc:mc + 1], in_=bkb[mc])

        def bcast(dram_ap):
            return bass.AP(tensor=dram_ap.tensor, offset=dram_ap.offset,
                           ap=[[0, P]] + list(dram_ap.ap[1:]))

        bv_bc = const.tile([P, F], bf16)
        nc.sync.dma_start(out=bv_bc[:], in_=bcast(bvb[:]))
        g_bc = const.tile([P, F], f32)
        b_bc = const.tile([P, F], f32)
        nc.sync.dma_start(out=g_bc[:], in_=bcast(lng[:]))
        nc.sync.dma_start(out=b_bc[:], in_=bcast(lnb[:]))

        xr_sb = const.tile([P, 4, F], f32)
        for qc in range(4):
            nc.sync.dma_start(out=xr_sb[:, qc, :], in_=xr[qc * P:(qc + 1) * P, :])

        w_sb = {}
        for nm, t in (("q", wqT), ("k", wkT), ("v", wvT)):
            w_sb[nm] = big.tile([P, 2, F], bf16, tag=f"w{nm}", name=f"w{nm}_sb")
            for kc in range(2):
                nc.sync.dma_start(out=w_sb[nm][:, kc, :], in_=t[kc])
        for kc in range(2):
            nc.sync.dma_start(out=xqT_sb[:, kc, :], in_=xqT[kc])
            nc.sync.dma_start(out=xT_sb[:, kc, :], in_=xT[kc])
        for kc in range(KC):
            nc.sync.dma_start(out=mmulT_sb[:, kc, :], in_=mmulT[kc])

        eps_sb = const.tile([P, 1], f32)
        nc.vector.memset(eps_sb[:], LN_EPS)
        rinv_sb = const.tile([P, H * 4], f32)
        out_sb = const.tile([P, 4, F], f32)
        srow = const.tile([1, H, NQ], f32)
        rr32 = const.tile([1, H, NQ], f32)
        rrbf = const.tile([1, H, NQ], bf16)

        # ---- projections ----
        for mc in range(2):
            ps = pmm.tile([P, 1024], f32, tag="mm", name="ps_q")
            for kc in range(2):
                nc.tensor.matmul(ps[:, 0:NQ],
                                 w_sb["q"][:, kc, mc * P:(mc + 1) * P],
                                 xqT_sb[:, kc, :],
                                 start=(kc == 0), stop=(kc == 1))
            nc.vector.tensor_scalar(out=QT_sb[:, mc, :], in0=ps[:, 0:NQ],
                                    scalar1=1.0 / np.sqrt(HD),
                                    scalar2=bq8_sb[:, mc:mc + 1],
                                    op0=OP.mult, op1=OP.add)
        def emit_kt_proj(mc, n2):
            ps = pmm.tile([P, 1024], f32, tag="mm", name="ps_k")
            for j in range(2):
                n0 = n2 * 1024 + j * 512
                for kc in range(2):
                    nc.tensor.matmul(ps[:, j * 512:(j + 1) * 512],
                                     w_sb["k"][:, kc, mc * P:(mc + 1) * P],
                                     xT_sb[:, kc, n0:n0 + 512],
                                     start=(kc == 0), stop=(kc == 1))
            nc.scalar.activation(
                out=KT_sb[:, mc, n2 * 1024:(n2 + 1) * 1024], in_=ps[:],
                func=AF.Identity, bias=bk_sb[:, mc:mc + 1], scale=1.0)

        def emit_v_proj(v4):
            # four V chunks (512 nodes) per PSUM tile
            ps = pmm.tile([P, 1024], f32, tag="mm", name="ps_v")
            for c in range(4):
                nc32 = 4 * v4 + c
                for kc in range(2):
                    nc.tensor.matmul(ps[:, c * F:(c + 1) * F],
                                     xT_sb[:, kc, nc32 * P:(nc32 + 1) * P],
                                     w_sb["v"][:, kc, :],
                                     start=(kc == 0), stop=(kc == 1))
            bv_ap = bv_bc[:]
            bv4 = bass.AP(tensor=bv_ap.tensor, offset=bv_ap.offset,
                          ap=[list(bv_ap.ap[0]), [0, 4], [HD, H], [1, HD]])
            nc.vector.scalar_tensor_tensor(
                out=V_sb[:, 4 * v4:4 * v4 + 4, :, 0:HD],
                in0=ps[:].rearrange("p (c h d) -> p c h d", c=4, h=H),
                scalar=0.0, in1=bv4, op0=OP.bypass, op1=OP.add)

        for mc in range(2):
            for n2 in range(4):
                emit_kt_proj(mc, n2)
        for v4 in range(8):
            emit_v_proj(v4)

        def qk_slice(t, h, lo, sz):
            return t[(h % 2) * HD:(h % 2 + 1) * HD, h // 2, lo:lo + sz]

        def emit_pass2_chunk(h, emc, rb, c8):
            # normalize one em chunk in place and DMA its attn blocks out
            rb_ap = rb[:]
            rb_rep = bass.AP(tensor=rb_ap.tensor, offset=rb_ap.offset,
                             ap=[list(rb_ap.ap[0]), [0, 2], list(rb_ap.ap[1])])
            for t4 in range(4):
                nc.vector.tensor_tensor(
                    out=emc[:, 2 * t4:2 * t4 + 2, :],
                    in0=emc[:, 2 * t4:2 * t4 + 2, :],
                    in1=rb_rep, op=OP.mult)
            nc.sync.dma_start(
                out=attn_d[h, 8 * c8:8 * (c8 + 1)].rearrange("c p q -> p c q"),
                in_=emc[:])

        prev = None       # (h, em_chunks, rb) of the previous head
        for h in range(H):
            # ---- pass 1: exp(scores^T) * mask -> em buffer + U~ accumulation
            # Software-pipelined: the U matmuls lag the score matmuls by
            # PIPE tiles so the PE never stalls on a fresh mask-multiply,
            # and the previous head's pass-2 chunks are interleaved in.
            PIPE = 3
            if prev is not None:
                for c8 in range(4):
                    emit_pass2_chunk(prev[0], prev[1][c8], prev[2], c8)
            up = pu.tile([HD1, NQ], f32, tag="u", name="up")
            em_chunks = []
            tiles = []    # per kc-pair: emc tile + slice index

            def emit_u(i):
                emc_i, t4_i = tiles[i]
                for j in range(2):
                    nc.tensor.matmul(up[:],
                                     V_sb[:, 2 * i + j, h, :],
                                     emc_i[:, 2 * t4_i + j, :],
                                     start=(2 * i + j == 0),
                                     stop=(2 * i + j == KC - 1))

            for i in range(16):          # kc pairs
                c8, t4 = i // 4, i % 4
                if t4 == 0:
                    emc = emh.tile([P, 8, NQ], bf16, tag="emh", name="emc")
                    em_chunks.append(emc)
                kc0 = 2 * i
                ps = pmm.tile([P, 1024], f32, tag="mm", name="ps_b")
                for j in range(2):
                    nc.tensor.matmul(ps[:, j * NQ:(j + 1) * NQ],
                                     qk_slice(KT_sb, h, (kc0 + j) * P, P),
                                     qk_slice(QT_sb, h, 0, NQ),
                                     start=True, stop=True)
                et = wk_b.tile([P, 1024], bf16, tag="et", name="et")
                nc.scalar.activation(out=et[:], in_=ps[:], func=AF.Exp)
                nc.vector.tensor_tensor(
                    out=emc[:, 2 * t4:2 * t4 + 2, :],
                    in0=et[:].rearrange("p (j q) -> p j q", j=2),
                    in1=mmulT_sb[:, kc0:kc0 + 2, :], op=OP.mult)
                tiles.append((emc, t4))
                if i >= PIPE:
                    emit_u(i - PIPE)
            for i in range(16 - PIPE, 16):
                emit_u(i)
            # ---- C: evacuate U~, transpose, rinv, scale V-aggregation
            us = wk_b.tile([HD1, NQ], f32, tag="us", name="us")
            nc.scalar.activation(out=us[:], in_=up[:], func=AF.Copy)
            tp = pu.tile([P, 4, HD1], f32, tag="u", name="tp")
            for qc in range(4):
                nc.tensor.transpose(tp[:, qc, :],
                                    us[:, qc * P:(qc + 1) * P], ident[:])
            for qc in range(4):
                idx = h * 4 + qc
                nc.vector.reciprocal(out=rinv_sb[:, idx:idx + 1],
                                     in_=tp[:, qc, HD:HD1])
                nc.vector.tensor_scalar_mul(
                    out_sb[:, qc, h * HD:(h + 1) * HD], tp[:, qc, 0:HD],
                    rinv_sb[:, idx:idx + 1])
            # rinv broadcast row: invert the sums row in place (stays on
            # partition 64), cast to bf16, then replicate across partitions
            # with a K=1 ones matmul on the PE
            nc.sync.dma_start(out=srow[:, h, :], in_=us[HD:HD1, :])
            nc.vector.reciprocal(out=rr32[:, h, :], in_=srow[:, h, :])
            nc.vector.tensor_copy(rrbf[:, h, :], rr32[:, h, :])
            rb = wk_b.tile([P, NQ], bf16, tag="rb", name="rb")
            nc.sync.dma_start(out=rr_dram[h:h + 1, :], in_=rrbf[:, h, :])
            nc.sync.dma_start(out=rb[:], in_=bcast(rr_dram[h:h + 1, :]))
            prev = (h, em_chunks, rb)

        # last head's pass 2
        for c8 in range(4):
            emit_pass2_chunk(prev[0], prev[1][c8], prev[2], c8)

        # ---- phase D: residual + layernorm + affine ----
        for qc in range(4):
            y = wk_d.tile([P, F], f32, tag="y", name="y")
            nc.vector.tensor_tensor(out=y[:], in0=out_sb[:, qc, :],
                                    in1=xr_sb[:, qc, :], op=OP.add)
            st = wk_d.tile([P, 6], f32, tag="st", name="st")
            nc.vector.bn_stats(out=st[:], in_=y[:])
            mv = wk_d.tile([P, 2], f32, tag="mv", name="mv")
            nc.vector.bn_aggr(out=mv[:], in_=st[:])
            sd = wk_d.tile([P, 1], f32, tag="sd", name="sd")
            nc.scalar.activation(out=sd[:], in_=mv[:, 1:2], func=AF.Sqrt,
                                 bias=eps_sb[:], scale=1.0)
            rs = wk_d.tile([P, 1], f32, tag="rs", name="rs")
            nc.vector.reciprocal(out=rs[:], in_=sd[:])
            yc = wk_d.tile([P, F], f32, tag="yc", name="yc")
            nc.vector.tensor_scalar(out=yc[:], in0=y[:],
                                    scalar1=mv[:, 0:1], scalar2=rs[:],
                                    op0=OP.subtract, op1=OP.mult)
            yg = wk_d.tile([P, F], f32, tag="yg", name="yg")
            nc.vector.tensor_tensor(out=yg[:], in0=yc[:], in1=g_bc[:],
                                    op=OP.mult)
            yo = wk_d.tile([P, F], f32, tag="yo", name="yo")
            nc.vector.tensor_tensor(out=yo[:], in0=yg[:], in1=b_bc[:],
                                    op=OP.add)
            nc.sync.dma_start(out=out_d[qc * P:(qc + 1) * P, :], in_=yo[:])

    nc.finalize()
    return nc


def _shard_inputs(inputs):
    import ml_dtypes
    bf = ml_dtypes.bfloat16

    x = np.asarray(inputs["x"], np.float32)
    ei = np.asarray(inputs["edge_index"], np.int64)
    Wq = np.asarray(inputs["Wq"], np.float32)
    bq = np.asarray(inputs["bq"], np.float32)
    Wk = np.asarray(inputs["Wk"], np.float32)
    bk = np.asarray(inputs["bk"], np.float32)
    Wv = np.asarray(inputs["Wv"], np.float32)
    bv = np.asarray(inputs["bv"], np.float32)
    ln_g = np.asarray(inputs["ln_g"], np.float32)
    ln_b = np.asarray(inputs["ln_b"], np.float32)

    adj = np.zeros((N, N), np.bool_)
    adj[ei[0], ei[1]] = True

    x0 = x[0]                                     # [N, F]
    xT = np.ascontiguousarray(x0.T)               # [F, N]

    def b16(a):
        return np.ascontiguousarray(a).astype(bf)

    xT_b = b16(xT).reshape(2, P, N)
    w = {nm: b16(W.T).reshape(2, P, F)
         for nm, W in (("q", Wq), ("k", Wk), ("v", Wv))}
    shared = {
        "xT": xT_b,
        "wqT": w["q"], "wkT": w["k"], "wvT": w["v"],
        "bq8": (bq / np.sqrt(HD)).astype(np.float32).reshape(2, P, 1),
        "bkb": bk.astype(np.float32).reshape(2, P, 1),
        "bvb": bv.reshape(1, F).astype(bf),
        "lng": ln_g.reshape(1, F).astype(np.float32),
        "lnb": ln_b.reshape(1, F).astype(np.float32),
    }
    in_maps = []
    for c in range(NCORES):
        rows = slice(c * NQ, (c + 1) * NQ)
        a = adj[rows]                             # [NQ, N]
        m = {
            "xqT": b16(xT[:, rows]).reshape(2, P, NQ),
            "xr": np.ascontiguousarray(x0[rows]).astype(np.float32),
            "mmulT": np.ascontiguousarray(a.T).astype(bf).reshape(KC, P, NQ),
        }
        m.update(shared)
        in_maps.append(m)
    return in_maps


def kernel(**inputs):
    global _BUILT, LAST_EXEC_NS, LAST_RESULTS
    from concourse.bass_utils import run_bass_kernel_spmd

    if _BUILT is None:
        _BUILT = _build()
    nc = _BUILT

    in_maps = _shard_inputs(inputs)
    res = run_bass_kernel_spmd(nc, in_maps, core_ids=list(range(NCORES)),
                               trace=TRACE)
    LAST_EXEC_NS = res.exec_time_ns
    LAST_RESULTS = res

    attn_full = np.empty((1, H, N, N), np.float32)
    out_full = np.empty((1, N, F), np.float32)
    for c in range(NCORES):
        rows = slice(c * NQ, (c + 1) * NQ)
        shard = np.asarray(res.results[c]["attn"])        # [H, KC, P, NQ]
        shard = shard.reshape(H, N, NQ).astype(np.float32)
        attn_full[0, :, rows, :] = shard.transpose(0, 2, 1)
        out_full[0, rows, :] = np.asarray(
            res.results[c]["outp"]).astype(np.float32)
    return out_full, attn_full


# revision 41
# speedup vs baseline: 1.0880x; 1.0086x over previous
"""nn_AdaptiveGraphLayer Trainium2 kernel (8 NeuronCores, SPMD).

Sharding: each core owns N/8 = 512 query rows for all H=4 heads.
 - x (node features) replicated -> every core computes K/V for all nodes.
 - adjacency mask built on host, sharded and transposed to [4096,512] (0/1
   multiplicative, kc-chunked).
 - attn is produced on device in a k-major blocked layout
   [H, 32, 128, 512] = [h, k-chunk, k, q] (fully normalized values); the
   host gather permutes it back to [H, 512, 4096]. out rows are f32.
   No device collectives.

Per-head device pipeline (single scores pass in the transposed layout):
  pass1: scores^T chunks via PE (K extended to 65 with a ones column ->
         row 64 of U~ accumulates the masked-exp row sums for free);
         ACT exp straight from PSUM (|scores| < ~4, safe unmasked);
         DVE mask-multiply (2x bf16) into a per-head em buffer;
         PE-accumulate U~ = [V_h | 1]^T @ em.
  C:     evacuate U~, PE-transpose, reciprocal of sums -> rinv;
         scale the V-aggregation; build a broadcast rinv row.
  pass2: normalize em in place (DVE 2x bf16) and DMA the attn blocks out.
  D:     residual + LayerNorm (bn_stats/bn_aggr), affine, DMA out rows.
"""

import numpy as np

B, N, F, H, HD = 1, 4096, 256, 4, 64
NCORES = 8
NQ = N // NCORES          # 512 query rows per core
P = 128
HD1 = HD + 1              # V plus the ones column
KC = 32                   # key chunks of 128
LN_EPS = 1e-5
GPS_MOD = 0               # every GPS_MOD-th mask-mult tile on GPSIMD (0=off)

TRACE = False             # set True (with ntff shim installed) to profile
LAST_EXEC_NS = None
LAST_RESULTS = None

_BUILT = None


def _build():
    from contextlib import ExitStack

    import concourse.bass as bass
    import concourse.bacc as bacc
    import concourse.mybir as mybir
    from concourse.tile import TileContext
    from concourse.masks import make_identity

    dt = mybir.dt
    f32, bf16 = dt.float32, dt.bfloat16
    AF = mybir.ActivationFunctionType
    OP = mybir.AluOpType

    nc = bacc.Bacc("TRN2", target_bir_lowering=False, debug=False,
                   num_devices=NCORES)

    def din(name, shape, dtype):
        return nc.declare_dram_parameter(name, list(shape), dtype, isOutput=False)

    def dout(name, shape, dtype):
        return nc.declare_dram_parameter(name, list(shape), dtype, isOutput=True)

    xT = din("xT", [2, P, N], bf16)          # x^T [f_in, node], f_in-chunked
    xqT = din("xqT", [2, P, NQ], bf16)       # this core's q columns of x^T
    xr = din("xr", [NQ, F], f32)             # residual rows (f32)
    wqT = din("wqT", [2, P, F], bf16)        # Wq.T [f_in, out], f_in-chunked
    wkT = din("wkT", [2, P, F], bf16)
    wvT = din("wvT", [2, P, F], bf16)
    bq8 = din("bq8", [2, P, 1], f32)         # bq / sqrt(hd)
    bkb = din("bkb", [2, P, 1], f32)
    bvb = din("bvb", [1, F], bf16)
    lng = din("lng", [1, F], f32)
    lnb = din("lnb", [1, F], f32)
    mmulT = din("mmulT", [KC, P, NQ], bf16)  # 0/1 mask^T, kc-chunked
    attn_d = dout("attn", [H, KC, P, NQ], bf16)   # [h, kc, k, q] blocks
    out_d = dout("outp", [NQ, F], f32)
    rr_dram = nc.dram_tensor("rr_scratch", [H, NQ], bf16)

    with TileContext(nc) as tc, ExitStack() as ctx:
        const = ctx.enter_context(tc.tile_pool(name="const", bufs=1))
        big = ctx.enter_context(tc.tile_pool(name="big", bufs=1))
        emh = ctx.enter_context(tc.tile_pool(name="emh", bufs=8))
        wk_b = ctx.enter_context(tc.tile_pool(name="wk_b", bufs=3))
        wk_d = ctx.enter_context(tc.tile_pool(name="wk_d", bufs=1))
        pmm = ctx.enter_context(tc.tile_pool(name="pmm", bufs=3, space="PSUM"))
        pu = ctx.enter_context(tc.tile_pool(name="pu", bufs=2, space="PSUM"))

        # ---- constants / parameters to SBUF ----
        ident = const.tile([HD1, HD1], f32)
        make_identity(nc, ident[:])
        ones1 = const.tile([1, P], bf16)
        nc.vector.memset(ones1[:], 1.0)
        ones_pad = const.tile([P, P], bf16)
        nc.vector.memset(ones_pad[:], 1.0)

        xT_sb = big.tile([P, 2, N], bf16)
        xqT_sb = big.tile([P, 2, NQ], bf16)
        KT_sb = big.tile([P, 2, N], bf16)
        QT_sb = big.tile([P, 2, NQ], bf16)
        V_sb = big.tile([P, KC, H, HD1], bf16)     # V plus ones column
        mmulT_sb = big.tile([P, KC, NQ], bf16)
        nc.vector.memset(V_sb[:, :, :, HD:HD1], 1.0)
        # small, projection-critical loads first; big masks last
        bq8_sb = const.tile([P, 2], f32)
        bk_sb = const.tile([P, 2], f32)
        for mc in range(2):
            nc.sync.dma_start(out=bq8_sb[:, mc:mc + 1], in_=bq8[mc])
            nc.sync.dma_start(out=bk_sb[:, mc:mc + 1], in_=bkb[mc])

        def bcast(dram_ap):
            return bass.AP(tensor=dram_ap.tensor, offset=dram_ap.offset,
                           ap=[[0, P]] + list(dram_ap.ap[1:]))

        bv_bc = const.tile([P, F], bf16)
        nc.sync.dma_start(out=bv_bc[:], in_=bcast(bvb[:]))
        g_bc = const.tile([P, F], f32)
        b_bc = const.tile([P, F], f32)
        nc.sync.dma_start(out=g_bc[:], in_=bcast(lng[:]))
        nc.sync.dma_start(out=b_bc[:], in_=bcast(lnb[:]))

        xr_sb = const.tile([P, 4, F], f32)
        for qc in range(4):
            nc.sync.dma_start(out=xr_sb[:, qc, :], in_=xr[qc * P:(qc + 1) * P, :])

        w_sb = {}
        for nm, t in (("q", wqT), ("k", wkT), ("v", wvT)):
            w_sb[nm] = big.tile([P, 2, F], bf16, tag=f"w{nm}", name=f"w{nm}_sb")
            for kc in range(2):
                nc.sync.dma_start(out=w_sb[nm][:, kc, :], in_=t[kc])
        for kc in range(2):
            nc.sync.dma_start(out=xqT_sb[:, kc, :], in_=xqT[kc])
            nc.sync.dma_start(out=xT_sb[:, kc, :], in_=xT[kc])
        for kc in range(KC):
            nc.sync.dma_start(out=mmulT_sb[:, kc, :], in_=mmulT[kc])

        eps_sb = const.tile([P, 1], f32)
        nc.vector.memset(eps_sb[:], LN_EPS)
        rinv_sb = const.tile([P, H * 4], f32)
        out_sb = const.tile([P, 4, F], f32)
        srow = const.tile([1, H, NQ], f32)
        rr32 = const.tile([1, H, NQ], f32)
        rrbf = const.tile([1, H, NQ], bf16)

        # ---- projections ----
        for mc in range(2):
            ps = pmm.tile([P, 1024], f32, tag="mm", name="ps_q")
            for kc in range(2):
                nc.tensor.matmul(ps[:, 0:NQ],
                                 w_sb["q"][:, kc, mc * P:(mc + 1) * P],
                                 xqT_sb[:, kc, :],
                                 start=(kc == 0), stop=(kc == 1))
            nc.vector.tensor_scalar(out=QT_sb[:, mc, :], in0=ps[:, 0:NQ],
                                    scalar1=1.0 / np.sqrt(HD),
                                    scalar2=bq8_sb[:, mc:mc + 1],
                                    op0=OP.mult, op1=OP.add)
        def emit_kt_proj(mc, n2):
            ps = pmm.tile([P, 1024], f32, tag="mm", name="ps_k")
            for j in range(2):
                n0 = n2 * 1024 + j * 512
                for kc in range(2):
                    nc.tensor.matmul(ps[:, j * 512:(j + 1) * 512],
                                     w_sb["k"][:, kc, mc * P:(mc + 1) * P],
                                     xT_sb[:, kc, n0:n0 + 512],
                                     start=(kc == 0), stop=(kc == 1))
            nc.scalar.activation(
                out=KT_sb[:, mc, n2 * 1024:(n2 + 1) * 1024], in_=ps[:],
                func=AF.Identity, bias=bk_sb[:, mc:mc + 1], scale=1.0)

        def emit_v_proj(v4):
            # four V chunks (512 nodes) per PSUM tile
            ps = pmm.tile([P, 1024], f32, tag="mm", name="ps_v")
            for c in range(4):
                nc32 = 4 * v4 + c
                for kc in range(2):
                    nc.tensor.matmul(ps[:, c * F:(c + 1) * F],
                                     xT_sb[:, kc, nc32 * P:(nc32 + 1) * P],
                                     w_sb["v"][:, kc, :],
                                     start=(kc == 0), stop=(kc == 1))
            bv_ap = bv_bc[:]
            bv4 = bass.AP(tensor=bv_ap.tensor, offset=bv_ap.offset,
                          ap=[list(bv_ap.ap[0]), [0, 4], [HD, H], [1, HD]])
            nc.vector.scalar_tensor_tensor(
                out=V_sb[:, 4 * v4:4 * v4 + 4, :, 0:HD],
                in0=ps[:].rearrange("p (c h d) -> p c h d", c=4, h=H),
                scalar=0.0, in1=bv4, op0=OP.bypass, op1=OP.add)

        for mc in range(2):
            for n2 in range(4):
                emit_kt_proj(mc, n2)
        for v4 in range(8):
            emit_v_proj(v4)

        def qk_slice(t, h, lo, sz):
            return t[(h % 2) * HD:(h % 2 + 1) * HD, h // 2, lo:lo + sz]

        def emit_pass2_chunk(h, emc, rb, c8):
            # normalize one em chunk in place and DMA its attn blocks out
            rb_ap = rb[:]
            rb_rep = bass.AP(tensor=rb_ap.tensor, offset=rb_ap.offset,
                             ap=[list(rb_ap.ap[0]), [0, 2], list(rb_ap.ap[1])])
            for t4 in range(4):
                nc.vector.tensor_tensor(
                    out=emc[:, 2 * t4:2 * t4 + 2, :],
                    in0=emc[:, 2 * t4:2 * t4 + 2, :],
                    in1=rb_rep, op=OP.mult)
            nc.sync.dma_start(
                out=attn_d[h, 8 * c8:8 * (c8 + 1)].rearrange("c p q -> p c q"),
                in_=emc[:])

        prev = None       # (h, em_chunks, rb) of the previous head
        for h in range(H):
            # ---- pass 1: exp(scores^T) * mask -> em buffer + U~ accumulation
            # Software-pipelined: the U matmuls lag the score matmuls by
            # PIPE tiles so the PE never stalls on a fresh mask-multiply,
            # and the previous head's pass-2 chunks are interleaved in.
            PIPE = 3
            if prev is not None:
                for c8 in range(4):
                    emit_pass2_chunk(prev[0], prev[1][c8], prev[2], c8)
            up = pu.tile([HD1, NQ], f32, tag="u", name="up")
            em_chunks = []
            tiles = []    # per kc-pair: emc tile + slice index

            def emit_u(i):
                emc_i, t4_i = tiles[i]
                for j in range(2):
                    nc.tensor.matmul(up[:],
                                     V_sb[:, 2 * i + j, h, :],
                                     emc_i[:, 2 * t4_i + j, :],
                                     start=(2 * i + j == 0),
                                     stop=(2 * i + j == KC - 1))

            for i in range(16):          # kc pairs
                c8, t4 = i // 4, i % 4
                if t4 == 0:
                    emc = emh.tile([P, 8, NQ], bf16, tag="emh", name="emc")
                    em_chunks.append(emc)
                kc0 = 2 * i
                ps = pmm.tile([P, 1024], f32, tag="mm", name="ps_b")
                for j in range(2):
                    nc.tensor.matmul(ps[:, j * NQ:(j + 1) * NQ],
                                     qk_slice(KT_sb, h, (kc0 + j) * P, P),
                                     qk_slice(QT_sb, h, 0, NQ),
                                     start=True, stop=True)
                et = wk_b.tile([P, 1024], bf16, tag="et", name="et")
                nc.scalar.activation(out=et[:], in_=ps[:], func=AF.Exp)
                nc.vector.tensor_tensor(
                    out=emc[:, 2 * t4:2 * t4 + 2, :],
                    in0=et[:].rearrange("p (j q) -> p j q", j=2),
                    in1=mmulT_sb[:, kc0:kc0 + 2, :], op=OP.mult)
                tiles.append((emc, t4))
                if i >= PIPE:
                    emit_u(i - PIPE)
            for i in range(16 - PIPE, 16):
                emit_u(i)
            # ---- C: evacuate U~, transpose, rinv, scale V-aggregation
            us = wk_b.tile([HD1, NQ], f32, tag="us", name="us")
            nc.scalar.activation(out=us[:], in_=up[:], func=AF.Copy)
            tp = pu.tile([P, 4, HD1], f32, tag="u", name="tp")
            for qc in range(4):
                nc.tensor.transpose(tp[:, qc, :],
                                    us[:, qc * P:(qc + 1) * P], ident[:])
            for qc in range(4):
                idx = h * 4 + qc
                nc.vector.reciprocal(out=rinv_sb[:, idx:idx + 1],
                                     in_=tp[:, qc, HD:HD1])
                nc.vector.tensor_scalar_mul(
                    out_sb[:, qc, h * HD:(h + 1) * HD], tp[:, qc, 0:HD],
                    rinv_sb[:, idx:idx + 1])
            # rinv broadcast row: invert the sums row in place (stays on
            # partition 64), cast to bf16, then replicate across partitions
            # with a K=1 ones matmul on the PE
            nc.sync.dma_start(out=srow[:, h, :], in_=us[HD:HD1, :])
            nc.vector.reciprocal(out=rr32[:, h, :], in_=srow[:, h, :])
            nc.vector.tensor_copy(rrbf[:, h, :], rr32[:, h, :])
            rb = wk_b.tile([P, NQ], bf16, tag="rb", name="rb")
            nc.sync.dma_start(out=rr_dram[h:h + 1, :], in_=rrbf[:, h, :])
            nc.sync.dma_start(out=rb[:], in_=bcast(rr_dram[h:h + 1, :]))
            prev = (h, em_chunks, rb)

        # last head's pass 2
        for c8 in range(4):
            emit_pass2_chunk(prev[0], prev[1][c8], prev[2], c8)

        # ---- phase D: residual + layernorm + affine ----
        for qc in range(4):
            y = wk_d.tile([P, F], f32, tag="y", name="y")
            nc.vector.tensor_tensor(out=y[:], in0=out_sb[:, qc, :],
                                    in1=xr_sb[:, qc, :], op=OP.add)
            st = wk_d.tile([P, 6], f32, tag="st", name="st")
            nc.vector.bn_stats(out=st[:], in_=y[:])
            mv = wk_d.tile([P, 2], f32, tag="mv", name="mv")
            nc.vector.bn_aggr(out=mv[:], in_=st[:])
            sd = wk_d.tile([P, 1], f32, tag="sd", name="sd")
            nc.scalar.activation(out=sd[:], in_=mv[:, 1:2], func=AF.Sqrt,
                                 bias=eps_sb[:], scale=1.0)
            rs = wk_d.tile([P, 1], f32, tag="rs", name="rs")
            nc.vector.reciprocal(out=rs[:], in_=sd[:])
            yc = wk_d.tile([P, F], f32, tag="yc", name="yc")
            nc.vector.tensor_scalar(out=yc[:], in0=y[:],
                                    scalar1=mv[:, 0:1], scalar2=rs[:],
                                    op0=OP.subtract, op1=OP.mult)
            yg = wk_d.tile([P, F], f32, tag="yg", name="yg")
            nc.vector.tensor_tensor(out=yg[:], in0=yc[:], in1=g_bc[:],
                                    op=OP.mult)
            yo = wk_d.tile([P, F], f32, tag="yo", name="yo")
            nc.vector.tensor_tensor(out=yo[:], in0=yg[:], in1=b_bc[:],
                                    op=OP.add)
            nc.sync.dma_start(out=out_d[qc * P:(qc + 1) * P, :], in_=yo[:])

    nc.finalize()
    return nc


def _shard_inputs(inputs):
    import ml_dtypes
    bf = ml_dtypes.bfloat16

    x = np.asarray(inputs["x"], np.float32)
    ei = np.asarray(inputs["edge_index"], np.int64)
    Wq = np.asarray(inputs["Wq"], np.float32)
    bq = np.asarray(inputs["bq"], np.float32)
    Wk = np.asarray(inputs["Wk"], np.float32)
    bk = np.asarray(inputs["bk"], np.float32)
    Wv = np.asarray(inputs["Wv"], np.float32)
    bv = np.asarray(inputs["bv"], np.float32)
    ln_g = np.asarray(inputs["ln_g"], np.float32)
    ln_b = np.asarray(inputs["ln_b"], np.float32)

    adj = np.zeros((N, N), np.bool_)
    adj[ei[0], ei[1]] = True

    x0 = x[0]                                     # [N, F]
    xT = np.ascontiguousarray(x0.T)               # [F, N]

    def b16(a):
        return np.ascontiguousarray(a).astype(bf)

    xT_b = b16(xT).reshape(2, P, N)
    w = {nm: b16(W.T).reshape(2, P, F)
         for nm, W in (("q", Wq), ("k", Wk), ("v", Wv))}
    shared = {
        "xT": xT_b,
        "wqT": w["q"], "wkT": w["k"], "wvT": w["v"],
        "bq8": (bq / np.sqrt(HD)).astype(np.float32).reshape(2, P, 1),
        "bkb": bk.astype(np.float32).reshape(2, P, 1),
        "bvb": bv.reshape(1, F).astype(bf),
        "lng": ln_g.reshape(1, F).astype(np.float32),
        "lnb": ln_b.reshape(1, F).astype(np.float32),
    }
    in_maps = []
    for c in range(NCORES):
        rows = slice(c * NQ, (c + 1) * NQ)
        a = adj[rows]                             # [NQ, N]
        m = {
            "xqT": b16(xT[:, rows]).reshape(2, P, NQ),
            "xr": np.ascontiguousarray(x0[rows]).astype(np.float32),
            "mmulT": np.ascontiguousarray(a.T).astype(bf).reshape(KC, P, NQ),
        }
        m.update(shared)
        in_maps.append(m)
    return in_maps


def kernel(**inputs):
    global _BUILT, LAST_EXEC_NS, LAST_RESULTS
    from concourse.bass_utils import run_bass_kernel_spmd

    if _BUILT is None:
        _BUILT = _build()
    nc = _BUILT

    in_maps = _shard_inputs(inputs)
    attn_full = np.empty((1, H, N, N), np.float32)
    out_full = np.empty((1, N, F), np.float32)
    for attempt in range(3):
        try:
            res = run_bass_kernel_spmd(nc, in_maps,
                                       core_ids=list(range(NCORES)),
                                       trace=TRACE)
            for c in range(NCORES):
                rows = slice(c * NQ, (c + 1) * NQ)
                shard = np.asarray(res.results[c]["attn"])    # [H, KC, P, NQ]
                shard = shard.reshape(H, N, NQ).astype(np.float32)
                attn_full[0, :, rows, :] = shard.transpose(0, 2, 1)
                out_full[0, rows, :] = np.asarray(
                    res.results[c]["outp"]).astype(np.float32)
            break
        except Exception:
            # transient NRT/device hiccups have been observed; retry
            if attempt == 2:
                raise
    LAST_EXEC_NS = res.exec_time_ns
    LAST_RESULTS = res
    return out_full, attn_full


# revision 43
# speedup vs baseline: 1.1143x; 1.0242x over previous
"""nn_AdaptiveGraphLayer Trainium2 kernel (8 NeuronCores, SPMD).

Sharding: each core owns N/8 = 512 query rows for all H=4 heads.
 - x (node features) replicated -> every core computes K/V for all nodes.
 - adjacency mask built on host, sharded and transposed to [4096,512] (0/1
   multiplicative, kc-chunked).
 - attn is produced on device in a k-major blocked layout
   [H, 32, 128, 512] = [h, k-chunk, k, q] (fully normalized values); the
   host gather permutes it back to [H, 512, 4096]. out rows are f32.
   No device collectives.

Per-head device pipeline (single scores pass in the transposed layout):
  pass1: scores^T chunks via PE (K extended to 65 with a ones column ->
         row 64 of U~ accumulates the masked-exp row sums for free);
         ACT exp straight from PSUM (|scores| < ~4, safe unmasked);
         DVE mask-multiply (2x bf16) into a per-head em buffer;
         PE-accumulate U~ = [V_h | 1]^T @ em.
  C:     evacuate U~, PE-transpose, reciprocal of sums -> rinv;
         scale the V-aggregation; build a broadcast rinv row.
  pass2: normalize em in place (DVE 2x bf16) and DMA the attn blocks out.
  D:     residual + LayerNorm (bn_stats/bn_aggr), affine, DMA out rows.
"""

import numpy as np

B, N, F, H, HD = 1, 4096, 256, 4, 64
NCORES = 8
NQ = N // NCORES          # 512 query rows per core
P = 128
HD1 = HD + 1              # V plus the ones column
KC = 32                   # key chunks of 128
LN_EPS = 1e-5
GPS_MOD = 0               # every GPS_MOD-th mask-mult tile on GPSIMD (0=off)

TRACE = False             # set True (with ntff shim installed) to profile
LAST_EXEC_NS = None
LAST_RESULTS = None

_BUILT = None


def _build():
    from contextlib import ExitStack

    import concourse.bass as bass
    import concourse.bacc as bacc
    import concourse.mybir as mybir
    from concourse.tile import TileContext
    from concourse.masks import make_identity

    dt = mybir.dt
    f32, bf16 = dt.float32, dt.bfloat16
    AF = mybir.ActivationFunctionType
    OP = mybir.AluOpType

    nc = bacc.Bacc("TRN2", target_bir_lowering=False, debug=False,
                   num_devices=NCORES)

    def din(name, shape, dtype):
        return nc.declare_dram_parameter(name, list(shape), dtype, isOutput=False)

    def dout(name, shape, dtype):
        return nc.declare_dram_parameter(name, list(shape), dtype, isOutput=True)

    xT = din("xT", [2, P, N], bf16)          # x^T [f_in, node], f_in-chunked
    xqT = din("xqT", [2, P, NQ], bf16)       # this core's q columns of x^T
    xr = din("xr", [NQ, F], f32)             # residual rows (f32)
    wqT = din("wqT", [2, P, F], bf16)        # Wq.T [f_in, out], f_in-chunked
    wkT = din("wkT", [2, P, F], bf16)
    wvT = din("wvT", [2, P, F], bf16)
    bq8 = din("bq8", [2, P, 1], f32)         # bq / sqrt(hd)
    bkb = din("bkb", [2, P, 1], f32)
    bvb = din("bvb", [1, F], bf16)
    lng = din("lng", [1, F], f32)
    lnb = din("lnb", [1, F], f32)
    mmulT = din("mmulT", [KC, P, NQ], bf16)  # 0/1 mask^T, kc-chunked
    attn_d = dout("attn", [H, KC, P, NQ], bf16)   # [h, kc, k, q] blocks
    out_d = dout("outp", [NQ, F], f32)
    rr_dram = nc.dram_tensor("rr_scratch", [H, NQ], bf16)

    with TileContext(nc) as tc, ExitStack() as ctx:
        const = ctx.enter_context(tc.tile_pool(name="const", bufs=1))
        big = ctx.enter_context(tc.tile_pool(name="big", bufs=1))
        emh = ctx.enter_context(tc.tile_pool(name="emh", bufs=8))
        wk_b = ctx.enter_context(tc.tile_pool(name="wk_b", bufs=3))
        wk_d = ctx.enter_context(tc.tile_pool(name="wk_d", bufs=1))
        pmm = ctx.enter_context(tc.tile_pool(name="pmm", bufs=3, space="PSUM"))
        pu = ctx.enter_context(tc.tile_pool(name="pu", bufs=2, space="PSUM"))

        # ---- constants / parameters to SBUF ----
        ident = const.tile([HD1, HD1], f32)
        make_identity(nc, ident[:])
        ones1 = const.tile([1, P], bf16)
        nc.vector.memset(ones1[:], 1.0)
        ones_pad = const.tile([P, P], bf16)
        nc.vector.memset(ones_pad[:], 1.0)

        xT_sb = big.tile([P, 2, N], bf16)
        xqT_sb = big.tile([P, 2, NQ], bf16)
        KT_sb = big.tile([P, 2, N], bf16)
        QT_sb = big.tile([P, 2, NQ], bf16)
        V_sb = big.tile([P, KC, H, HD1], bf16)     # V plus ones column
        mmulT_sb = big.tile([P, KC, NQ], bf16)
        nc.vector.memset(V_sb[:, :, :, HD:HD1], 1.0)
        # small, projection-critical loads first; big masks last
        bq8_sb = const.tile([P, 2], f32)
        bk_sb = const.tile([P, 2], f32)
        for mc in range(2):
            nc.sync.dma_start(out=bq8_sb[:, mc:mc + 1], in_=bq8[mc])
            nc.sync.dma_start(out=bk_sb[:, mc:mc + 1], in_=bkb[mc])

        def bcast(dram_ap):
            return bass.AP(tensor=dram_ap.tensor, offset=dram_ap.offset,
                           ap=[[0, P]] + list(dram_ap.ap[1:]))

        bv_bc = const.tile([P, F], bf16)
        nc.sync.dma_start(out=bv_bc[:], in_=bcast(bvb[:]))
        g_bc = const.tile([P, F], f32)
        b_bc = const.tile([P, F], f32)
        nc.sync.dma_start(out=g_bc[:], in_=bcast(lng[:]))
        nc.sync.dma_start(out=b_bc[:], in_=bcast(lnb[:]))

        xr_sb = const.tile([P, 4, F], f32)
        for qc in range(4):
            nc.sync.dma_start(out=xr_sb[:, qc, :], in_=xr[qc * P:(qc + 1) * P, :])

        w_sb = {}
        for nm, t in (("q", wqT), ("k", wkT), ("v", wvT)):
            w_sb[nm] = big.tile([P, 2, F], bf16, tag=f"w{nm}", name=f"w{nm}_sb")
            for kc in range(2):
                nc.sync.dma_start(out=w_sb[nm][:, kc, :], in_=t[kc])
        for kc in range(2):
            nc.sync.dma_start(out=xqT_sb[:, kc, :], in_=xqT[kc])
        # xT split by key-range so the KT projection can start early
        for n2 in range(4):
            for kc in range(2):
                nc.sync.dma_start(
                    out=xT_sb[:, kc, n2 * 1024:(n2 + 1) * 1024],
                    in_=xT[kc, :, n2 * 1024:(n2 + 1) * 1024])
        for kc in range(KC):
            nc.sync.dma_start(out=mmulT_sb[:, kc, :], in_=mmulT[kc])

        eps_sb = const.tile([P, 1], f32)
        nc.vector.memset(eps_sb[:], LN_EPS)
        rinv_sb = const.tile([P, H * 4], f32)
        out_sb = const.tile([P, 4, F], f32)
        srow = const.tile([1, H, NQ], f32)
        rr32 = const.tile([1, H, NQ], f32)
        rrbf = const.tile([1, H, NQ], bf16)

        # ---- projections ----
        for mc in range(2):
            ps = pmm.tile([P, 1024], f32, tag="mm", name="ps_q")
            for kc in range(2):
                nc.tensor.matmul(ps[:, 0:NQ],
                                 w_sb["q"][:, kc, mc * P:(mc + 1) * P],
                                 xqT_sb[:, kc, :],
                                 start=(kc == 0), stop=(kc == 1))
            nc.vector.tensor_scalar(out=QT_sb[:, mc, :], in0=ps[:, 0:NQ],
                                    scalar1=1.0 / np.sqrt(HD),
                                    scalar2=bq8_sb[:, mc:mc + 1],
                                    op0=OP.mult, op1=OP.add)
        def emit_kt_proj(mc, n2):
            ps = pmm.tile([P, 1024], f32, tag="mm", name="ps_k")
            for j in range(2):
                n0 = n2 * 1024 + j * 512
                for kc in range(2):
                    nc.tensor.matmul(ps[:, j * 512:(j + 1) * 512],
                                     w_sb["k"][:, kc, mc * P:(mc + 1) * P],
                                     xT_sb[:, kc, n0:n0 + 512],
                                     start=(kc == 0), stop=(kc == 1))
            nc.scalar.activation(
                out=KT_sb[:, mc, n2 * 1024:(n2 + 1) * 1024], in_=ps[:],
                func=AF.Identity, bias=bk_sb[:, mc:mc + 1], scale=1.0)

        def emit_v_proj(v4):
            # four V chunks (512 nodes) per PSUM tile
            ps = pmm.tile([P, 1024], f32, tag="mm", name="ps_v")
            for c in range(4):
                nc32 = 4 * v4 + c
                for kc in range(2):
                    nc.tensor.matmul(ps[:, c * F:(c + 1) * F],
                                     xT_sb[:, kc, nc32 * P:(nc32 + 1) * P],
                                     w_sb["v"][:, kc, :],
                                     start=(kc == 0), stop=(kc == 1))
            bv_ap = bv_bc[:]
            bv4 = bass.AP(tensor=bv_ap.tensor, offset=bv_ap.offset,
                          ap=[list(bv_ap.ap[0]), [0, 4], [HD, H], [1, HD]])
            nc.vector.scalar_tensor_tensor(
                out=V_sb[:, 4 * v4:4 * v4 + 4, :, 0:HD],
                in0=ps[:].rearrange("p (c h d) -> p c h d", c=4, h=H),
                scalar=0.0, in1=bv4, op0=OP.bypass, op1=OP.add)

        for mc in range(2):
            for n2 in range(4):
                emit_kt_proj(mc, n2)
        for v4 in range(8):
            emit_v_proj(v4)

        def qk_slice(t, h, lo, sz):
            return t[(h % 2) * HD:(h % 2 + 1) * HD, h // 2, lo:lo + sz]

        def emit_pass2_chunk(h, emc, rb, c8):
            # normalize one em chunk in place (single wide 2x TT) and DMA
            # its attn blocks out
            rb_ap = rb[:]
            rb_rep = bass.AP(tensor=rb_ap.tensor, offset=rb_ap.offset,
                             ap=[list(rb_ap.ap[0]), [0, 8], list(rb_ap.ap[1])])
            nc.vector.tensor_tensor(out=emc[:], in0=emc[:], in1=rb_rep,
                                    op=OP.mult)
            nc.sync.dma_start(
                out=attn_d[h, 8 * c8:8 * (c8 + 1)].rearrange("c p q -> p c q"),
                in_=emc[:])

        prev = None       # (h, em_chunks, rb) of the previous head
        for h in range(H):
            # ---- pass 1: exp(scores^T) * mask -> em buffer + U~ accumulation
            # Software-pipelined: the U matmuls lag the score matmuls by
            # PIPE tiles so the PE never stalls on a fresh mask-multiply,
            # and the previous head's pass-2 chunks are interleaved in.
            PIPE = 3
            if prev is not None:
                for c8 in range(4):
                    emit_pass2_chunk(prev[0], prev[1][c8], prev[2], c8)
            up = pu.tile([HD1, NQ], f32, tag="u", name="up")
            em_chunks = []
            tiles = []    # per kc-pair: emc tile + slice index

            def emit_u(i):
                emc_i, t4_i = tiles[i]
                for j in range(2):
                    nc.tensor.matmul(up[:],
                                     V_sb[:, 2 * i + j, h, :],
                                     emc_i[:, 2 * t4_i + j, :],
                                     start=(2 * i + j == 0),
                                     stop=(2 * i + j == KC - 1))

            for i in range(16):          # kc pairs
                c8, t4 = i // 4, i % 4
                if t4 == 0:
                    emc = emh.tile([P, 8, NQ], bf16, tag="emh", name="emc")
                    em_chunks.append(emc)
                kc0 = 2 * i
                ps = pmm.tile([P, 1024], f32, tag="mm", name="ps_b")
                for j in range(2):
                    nc.tensor.matmul(ps[:, j * NQ:(j + 1) * NQ],
                                     qk_slice(KT_sb, h, (kc0 + j) * P, P),
                                     qk_slice(QT_sb, h, 0, NQ),
                                     start=True, stop=True)
                et = wk_b.tile([P, 1024], bf16, tag="et", name="et")
                nc.scalar.activation(out=et[:], in_=ps[:], func=AF.Exp)
                nc.vector.tensor_tensor(
                    out=emc[:, 2 * t4:2 * t4 + 2, :],
                    in0=et[:].rearrange("p (j q) -> p j q", j=2),
                    in1=mmulT_sb[:, kc0:kc0 + 2, :], op=OP.mult)
                tiles.append((emc, t4))
                if i >= PIPE:
                    emit_u(i - PIPE)
            for i in range(16 - PIPE, 16):
                emit_u(i)
            # ---- C: evacuate U~, transpose, rinv, scale V-aggregation
            us = wk_b.tile([HD1, NQ], f32, tag="us", name="us")
            nc.scalar.activation(out=us[:], in_=up[:], func=AF.Copy)
            tp = pu.tile([P, 4, HD1], f32, tag="u", name="tp")
            for qc in range(4):
                nc.tensor.transpose(tp[:, qc, :],
                                    us[:, qc * P:(qc + 1) * P], ident[:])
            for qc in range(4):
                idx = h * 4 + qc
                nc.vector.reciprocal(out=rinv_sb[:, idx:idx + 1],
                                     in_=tp[:, qc, HD:HD1])
                nc.vector.tensor_scalar_mul(
                    out_sb[:, qc, h * HD:(h + 1) * HD], tp[:, qc, 0:HD],
                    rinv_sb[:, idx:idx + 1])
            # rinv broadcast row: invert the sums row in place (stays on
            # partition 64), cast to bf16, then replicate across partitions
            # with a K=1 ones matmul on the PE
            nc.sync.dma_start(out=srow[:, h, :], in_=us[HD:HD1, :])
            nc.vector.reciprocal(out=rr32[:, h, :], in_=srow[:, h, :])
            nc.vector.tensor_copy(rrbf[:, h, :], rr32[:, h, :])
            rb = wk_b.tile([P, NQ], bf16, tag="rb", name="rb")
            nc.sync.dma_start(out=rr_dram[h:h + 1, :], in_=rrbf[:, h, :])
            nc.sync.dma_start(out=rb[:], in_=bcast(rr_dram[h:h + 1, :]))
            prev = (h, em_chunks, rb)

        # last head's pass 2
        for c8 in range(4):
            emit_pass2_chunk(prev[0], prev[1][c8], prev[2], c8)

        # ---- phase D: residual + layernorm + affine ----
        for qc in range(4):
            y = wk_d.tile([P, F], f32, tag="y", name="y")
            nc.vector.tensor_tensor(out=y[:], in0=out_sb[:, qc, :],
                                    in1=xr_sb[:, qc, :], op=OP.add)
            st = wk_d.tile([P, 6], f32, tag="st", name="st")
            nc.vector.bn_stats(out=st[:], in_=y[:])
            mv = wk_d.tile([P, 2], f32, tag="mv", name="mv")
            nc.vector.bn_aggr(out=mv[:], in_=st[:])
            sd = wk_d.tile([P, 1], f32, tag="sd", name="sd")
            nc.scalar.activation(out=sd[:], in_=mv[:, 1:2], func=AF.Sqrt,
                                 bias=eps_sb[:], scale=1.0)
            rs = wk_d.tile([P, 1], f32, tag="rs", name="rs")
            nc.vector.reciprocal(out=rs[:], in_=sd[:])
            yc = wk_d.tile([P, F], f32, tag="yc", name="yc")
            nc.vector.tensor_scalar(out=yc[:], in0=y[:],
                                    scalar1=mv[:, 0:1], scalar2=rs[:],
                                    op0=OP.subtract, op1=OP.mult)
            yg = wk_d.tile([P, F], f32, tag="yg", name="yg")
            nc.vector.tensor_tensor(out=yg[:], in0=yc[:], in1=g_bc[:],
                                    op=OP.mult)
            yo = wk_d.tile([P, F], f32, tag="yo", name="yo")
            nc.vector.tensor_tensor(out=yo[:], in0=yg[:], in1=b_bc[:],
                                    op=OP.add)
            nc.sync.dma_start(out=out_d[qc * P:(qc + 1) * P, :], in_=yo[:])

    nc.finalize()
    return nc


def _shard_inputs(inputs):
    import ml_dtypes
    bf = ml_dtypes.bfloat16

    x = np.asarray(inputs["x"], np.float32)
    ei = np.asarray(inputs["edge_index"], np.int64)
    Wq = np.asarray(inputs["Wq"], np.float32)
    bq = np.asarray(inputs["bq"], np.float32)
    Wk = np.asarray(inputs["Wk"], np.float32)
    bk = np.asarray(inputs["bk"], np.float32)
    Wv = np.asarray(inputs["Wv"], np.float32)
    bv = np.asarray(inputs["bv"], np.float32)
    ln_g = np.asarray(inputs["ln_g"], np.float32)
    ln_b = np.asarray(inputs["ln_b"], np.float32)

    adj = np.zeros((N, N), np.bool_)
    adj[ei[0], ei[1]] = True

    x0 = x[0]                                     # [N, F]
    xT = np.ascontiguousarray(x0.T)               # [F, N]

    def b16(a):
        return np.ascontiguousarray(a).astype(bf)

    xT_b = b16(xT).reshape(2, P, N)
    w = {nm: b16(W.T).reshape(2, P, F)
         for nm, W in (("q", Wq), ("k", Wk), ("v", Wv))}
    shared = {
        "xT": xT_b,
        "wqT": w["q"], "wkT": w["k"], "wvT": w["v"],
        "bq8": (bq / np.sqrt(HD)).astype(np.float32).reshape(2, P, 1),
        "bkb": bk.astype(np.float32).reshape(2, P, 1),
        "bvb": bv.reshape(1, F).astype(bf),
        "lng": ln_g.reshape(1, F).astype(np.float32),
        "lnb": ln_b.reshape(1, F).astype(np.float32),
    }
    in_maps = []
    for c in range(NCORES):
        rows = slice(c * NQ, (c + 1) * NQ)
        a = adj[rows]                             # [NQ, N]
        m = {
            "xqT": b16(xT[:, rows]).reshape(2, P, NQ),
            "xr": np.ascontiguousarray(x0[rows]).astype(np.float32),
            "mmulT": np.ascontiguousarray(a.T).astype(bf).reshape(KC, P, NQ),
        }
        m.update(shared)
        in_maps.append(m)
    return in_maps


def kernel(**inputs):
    global _BUILT, LAST_EXEC_NS, LAST_RESULTS
    from concourse.bass_utils import run_bass_kernel_spmd

    if _BUILT is None:
        _BUILT = _build()
    nc = _BUILT

    in_maps = _shard_inputs(inputs)
    attn_full = np.empty((1, H, N, N), np.float32)
    out_full = np.empty((1, N, F), np.float32)
    for attempt in range(3):
        try:
            res = run_bass_kernel_spmd(nc, in_maps,
                                       core_ids=list(range(NCORES)),
                                       trace=TRACE)
            for c in range(NCORES):
                rows = slice(c * NQ, (c + 1) * NQ)
                shard = np.asarray(res.results[c]["attn"])    # [H, KC, P, NQ]
                shard = shard.reshape(H, N, NQ).astype(np.float32)
                attn_full[0, :, rows, :] = shard.transpose(0, 2, 1)
                out_full[0, rows, :] = np.asarray(
                    res.results[c]["outp"]).astype(np.float32)
            break
        except Exception:
            # transient NRT/device hiccups have been observed; retry
            if attempt == 2:
                raise
    LAST_EXEC_NS = res.exec_time_ns
    LAST_RESULTS = res
    return out_full, attn_full
